# revision 1
# baseline (speedup 1.0000x reference)
"""Trainium2 Bass kernel: 7x7 valid cross-correlation + bias on a 4096x4096 f32 image.

Formulation: banded matmul on the TensorEngine.
  out[r, c] = sum_{di,dj} w[di,dj] * x[r+di, c+dj]
For an output row-strip of M=122 rows starting at r0, using K=128 input rows:
  out[r0+m, c] = sum_k A_dj[k, m] * x[r0+k, c+dj]   summed over dj=0..6
where A_dj[k, m] = w[k-m, dj] for 0 <= k-m < 7 (a banded [128, 122] matrix,
precomputed on host from the 49 kernel weights). The 7 dj-terms accumulate
into one PSUM bank via shifted column slices of the same SBUF rhs tile.

Weight-grouped schedule: G strips are processed together with dj as the
outer loop, so G consecutive matmuls (G strips x G PSUM banks) share the same
stationary weights — weight reloads break the PE pipeline (~250 ns each), so
amortizing them Gx matters.

Sharding: output columns are split across the 8 cores (512 cols/core);
each core processes all 4090 output rows. Kernel + bias replicated.
"""

import numpy as np

H, W = 4096, 4096
KH, KW = 7, 7
OH, OW = H - KH + 1, W - KW + 1  # 4090, 4090
N_CORES = 8
CW = 512               # output columns per core
IW = CW + KW - 1       # input columns per core (518)
STRIP = 122            # output rows per strip (K = STRIP + KH - 1 = 128)
MB = 128               # stationary block columns (M padded 122 -> 128)
N_STRIPS = (OH + STRIP - 1) // STRIP  # 34 (last strip M=64, K=70)
G = 4                  # strips per weight-group
SCHEDULE = "grouped"   # "grouped" (G-strip weight groups) | "djouter" (dj global outer)

_cache = {}


def _build_nc_djouter():
    """dj as the global outer loop: one stationary load per dj for the whole
    kernel (7 total). Each matmul is a single-MM PSUM group (start+stop) whose
    result the DVE folds into a per-strip SBUF accumulator. All 34 rhs tiles
    and accumulators stay SBUF-resident (~144KB/partition)."""
    import concourse.bacc as bacc
    import concourse.mybir as mybir
    from concourse.tile import TileContext

    f32 = mybir.dt.float32

    nc = bacc.Bacc("TRN2", target_bir_lowering=False, debug=False)
    xs = nc.dram_tensor("xs", [H, IW], f32, kind="ExternalInput")
    bands = nc.dram_tensor("bands", [128, KW * MB], f32, kind="ExternalInput")
    biasv = nc.dram_tensor("biasv", [128, 1], f32, kind="ExternalInput")
    out = nc.dram_tensor("out", [OH, CW], f32, kind="ExternalOutput")

    with TileContext(nc) as tc:
        with (
            tc.tile_pool(name="const", bufs=1) as cpool,
            tc.tile_pool(name="rhs", bufs=N_STRIPS) as rpool,
            tc.tile_pool(name="acc", bufs=N_STRIPS) as apool,
            tc.tile_pool(name="psum", bufs=8, space="PSUM") as ppool,
        ):
            band_t = cpool.tile([128, KW * MB], f32)
            nc.sync.dma_start(out=band_t[:, :], in_=bands[:, :])
            bias_t = cpool.tile([128, 1], f32)
            nc.sync.dma_start(out=bias_t[:, :], in_=biasv[:, :])

            dims = []
            rhs_ts, acc_ts = [], []
            for s in range(N_STRIPS):
                r0 = s * STRIP
                M = min(STRIP, OH - r0)
                K = min(128, H - r0)
                dims.append((r0, M, K))
                rhs_t = rpool.tile([128, IW], f32, name="rhs", tag="rhs")
                nc.sync.dma_start(out=rhs_t[:K, :], in_=xs[r0 : r0 + K, :])
                rhs_ts.append(rhs_t)
                acc_ts.append(apool.tile([128, CW], f32, name="acc", tag="acc"))

            for dj in range(KW):
                lhsT = band_t[:, dj * MB : dj * MB + MB]
                for s in range(N_STRIPS):
                    r0, M, K = dims[s]
                    ps = ppool.tile([128, CW], f32, name="ps", tag="ps")
                    nc.tensor.matmul(
                        ps[:, :],
                        lhsT[:K, :],
                        rhs_ts[s][:K, dj : dj + CW],
                        start=True,
                        stop=True,
                    )
                    if dj == 0:
                        # init: acc = psum + bias
                        nc.vector.tensor_scalar_add(
                            acc_ts[s][:M, :], ps[:M, :], bias_t[:M, :1]
                        )
                    else:
                        nc.vector.tensor_add(
                            acc_ts[s][:M, :], acc_ts[s][:M, :], ps[:M, :]
                        )

            for s in range(N_STRIPS):
                r0, M, K = dims[s]
                nc.sync.dma_start(out=out[r0 : r0 + M, :], in_=acc_ts[s][:M, :])

    nc.finalize()
    return nc


def _build_nc():
    if SCHEDULE == "djouter":
        return _build_nc_djouter()
    import concourse.bacc as bacc
    import concourse.mybir as mybir
    from concourse.tile import TileContext

    f32 = mybir.dt.float32

    nc = bacc.Bacc("TRN2", target_bir_lowering=False, debug=False)
    xs = nc.dram_tensor("xs", [H, IW], f32, kind="ExternalInput")
    bands = nc.dram_tensor("bands", [128, KW * MB], f32, kind="ExternalInput")
    biasv = nc.dram_tensor("biasv", [128, 1], f32, kind="ExternalInput")
    out = nc.dram_tensor("out", [OH, CW], f32, kind="ExternalOutput")

    with TileContext(nc) as tc:
        with (
            tc.tile_pool(name="const", bufs=1) as cpool,
            tc.tile_pool(name="rhs", bufs=2 * G) as rpool,
            tc.tile_pool(name="obuf", bufs=2 * G) as opool,
            tc.tile_pool(name="psum", bufs=8, space="PSUM") as ppool,
        ):
            band_t = cpool.tile([128, KW * MB], f32)
            nc.sync.dma_start(out=band_t[:, :], in_=bands[:, :])
            bias_t = cpool.tile([128, 1], f32)
            nc.sync.dma_start(out=bias_t[:, :], in_=biasv[:, :])

            for g0 in range(0, N_STRIPS, G):
                strips = [s for s in range(g0, min(g0 + G, N_STRIPS))]
                rhs_ts, ps_ts, dims = [], [], []
                for s in strips:
                    r0 = s * STRIP
                    M = min(STRIP, OH - r0)
                    K = min(128, H - r0)
                    rhs_t = rpool.tile([128, IW], f32, tag="rhs")
                    nc.sync.dma_start(out=rhs_t[:K, :], in_=xs[r0 : r0 + K, :])
                    rhs_ts.append(rhs_t)
                    ps_ts.append(ppool.tile([128, CW], f32, name="ps", tag="ps"))
                    dims.append((r0, M, K))
                for dj in range(KW):
                    lhsT = band_t[:, dj * MB : dj * MB + MB]
                    for rhs_t, ps, (r0, M, K) in zip(rhs_ts, ps_ts, dims):
                        nc.tensor.matmul(
                            ps[:, :],
                            lhsT[:K, :],
                            rhs_t[:K, dj : dj + CW],
                            start=(dj == 0),
                            stop=(dj == KW - 1),
                        )
                for ps, (r0, M, K) in zip(ps_ts, dims):
                    ot = opool.tile([128, CW], f32, tag="ot")
                    nc.vector.tensor_scalar_add(ot[:M, :], ps[:M, :], bias_t[:M, :1])
                    nc.sync.dma_start(out=out[r0 : r0 + M, :], in_=ot[:M, :])

    nc.finalize()
    return nc


def _get_nc():
    if "nc" not in _cache:
        _cache["nc"] = _build_nc()
    return _cache["nc"]


def _build_bands(weight: np.ndarray) -> np.ndarray:
    """bands[k, dj*MB + m] = weight[k - m, dj] for 0 <= k-m < KH, m < STRIP."""
    w = np.asarray(weight, np.float32)
    bands = np.zeros((128, KW * MB), np.float32)
    m = np.arange(STRIP)
    for dj in range(KW):
        for di in range(KH):
            bands[m + di, dj * MB + m] = w[di, dj]
    return bands


def _prepare_in_maps(x, weight, bias):
    x = np.ascontiguousarray(x, np.float32)
    bands = _build_bands(weight)
    bias_tile = np.full((128, 1), np.float32(np.asarray(bias).reshape(-1)[0]))

    in_maps = []
    for c in range(N_CORES):
        c0 = c * CW
        avail = min(IW, W - c0)
        if avail == IW:
            xs = x[:, c0 : c0 + IW]
        else:
            xs = np.zeros((H, IW), np.float32)
            xs[:, :avail] = x[:, c0 : c0 + avail]
        in_maps.append({"xs": xs, "bands": bands, "biasv": bias_tile})
    return in_maps


def _gather_out(per_core_outs) -> np.ndarray:
    out = np.empty((OH, OW), np.float32)
    for c in range(N_CORES):
        c0 = c * CW
        take = min(CW, OW - c0)
        out[:, c0 : c0 + take] = per_core_outs[c]["out"][:, :take]
    return out


def kernel(x: np.ndarray, weight: np.ndarray, bias: np.ndarray) -> np.ndarray:
    from concourse import bass_utils

    nc = _get_nc()
    in_maps = _prepare_in_maps(x, weight, bias)
    res = bass_utils.run_bass_kernel_spmd(nc, in_maps, list(range(N_CORES)))
    _cache["last_results"] = res
    return _gather_out(res.results)



# revision 2
# speedup vs baseline: 1.8872x; 1.8872x over previous
"""Trainium2 Bass kernel: 7x7 valid cross-correlation + bias on a 4096x4096 f32 image.

Formulation: banded matmul on the TensorEngine.
  out[r, c] = sum_{di,dj} w[di,dj] * x[r+di, c+dj]
For an output row-strip of M=122 rows starting at r0, using K=128 input rows:
  out[r0+m, c] = sum_k A_dj[k, m] * x[r0+k, c+dj]   summed over dj=0..6
where A_dj[k, m] = w[k-m, dj] for 0 <= k-m < 7 (a banded [128, 122] matrix,
precomputed on host from the 49 kernel weights). The 7 dj-terms accumulate
into one PSUM bank via shifted column slices of the same SBUF rhs tile.

All matmul operands are fp16: the PE streams 16-bit data at 1 cycle/row vs
fp32's LOW_HIGH two-pass mode (4 cycles/row + doubled LDWEIGHTS), and input
DMA halves. PSUM accumulation stays fp32; the output is staged to SBUF as
fp16 (bias added by the DVE) and upcast to fp32 on the host.

Weight-grouped schedule: G strips are processed together with dj as the
outer loop, so G consecutive matmuls (G strips x G PSUM banks) share the same
stationary weights.

Sharding: output columns are split across the 8 cores (512 cols/core);
each core processes all 4090 output rows. Kernel + bias replicated.
"""

import numpy as np

H, W = 4096, 4096
KH, KW = 7, 7
OH, OW = H - KH + 1, W - KW + 1  # 4090, 4090
N_CORES = 8
CW = 512               # output columns per core
IW = CW + KW - 1       # input columns per core (518)
STRIP = 122            # output rows per strip (K = STRIP + KH - 1 = 128)
MB = 128               # stationary block columns (M padded 122 -> 128)
N_STRIPS = (OH + STRIP - 1) // STRIP  # 34 (last strip M=64, K=70)
G = 4                  # strips per weight-group

_cache = {}


def _build_nc():
    import concourse.bacc as bacc
    import concourse.mybir as mybir
    from concourse.tile import TileContext

    f16 = mybir.dt.float16
    f32 = mybir.dt.float32

    nc = bacc.Bacc("TRN2", target_bir_lowering=False, debug=False)
    xs = nc.dram_tensor("xs", [H, IW], f16, kind="ExternalInput")
    bands = nc.dram_tensor("bands", [128, KW * MB], f16, kind="ExternalInput")
    biasv = nc.dram_tensor("biasv", [128, 1], f32, kind="ExternalInput")
    out = nc.dram_tensor("out", [OH, CW], f16, kind="ExternalOutput")

    with TileContext(nc) as tc:
        with (
            tc.tile_pool(name="const", bufs=1) as cpool,
            tc.tile_pool(name="rhs", bufs=2 * G) as rpool,
            tc.tile_pool(name="obuf", bufs=2 * G) as opool,
            tc.tile_pool(name="psum", bufs=8, space="PSUM") as ppool,
        ):
            band_t = cpool.tile([128, KW * MB], f16)
            nc.sync.dma_start(out=band_t[:, :], in_=bands[:, :])
            bias_t = cpool.tile([128, 1], f32)
            nc.sync.dma_start(out=bias_t[:, :], in_=biasv[:, :])

            for g0 in range(0, N_STRIPS, G):
                strips = [s for s in range(g0, min(g0 + G, N_STRIPS))]
                rhs_ts, ps_ts, dims = [], [], []
                for s in strips:
                    r0 = s * STRIP
                    M = min(STRIP, OH - r0)
                    K = min(128, H - r0)
                    rhs_t = rpool.tile([128, IW], f16, tag="rhs")
                    nc.sync.dma_start(out=rhs_t[:K, :], in_=xs[r0 : r0 + K, :])
                    rhs_ts.append(rhs_t)
                    ps_ts.append(ppool.tile([128, CW], f32, name="ps", tag="ps"))
                    dims.append((r0, M, K))
                for dj in range(KW):
                    lhsT = band_t[:, dj * MB : dj * MB + MB]
                    for rhs_t, ps, (r0, M, K) in zip(rhs_ts, ps_ts, dims):
                        nc.tensor.matmul(
                            ps[:, :],
                            lhsT[:K, :],
                            rhs_t[:K, dj : dj + CW],
                            start=(dj == 0),
                            stop=(dj == KW - 1),
                        )
                for ps, (r0, M, K) in zip(ps_ts, dims):
                    ot = opool.tile([128, CW], f16, tag="ot")
                    nc.vector.tensor_scalar_add(ot[:M, :], ps[:M, :], bias_t[:M, :1])
                    nc.sync.dma_start(out=out[r0 : r0 + M, :], in_=ot[:M, :])

    nc.finalize()
    return nc


def _get_nc():
    if "nc" not in _cache:
        _cache["nc"] = _build_nc()
    return _cache["nc"]


def _build_bands(weight: np.ndarray) -> np.ndarray:
    """bands[k, dj*MB + m] = weight[k - m, dj] for 0 <= k-m < KH, m < STRIP."""
    w = np.asarray(weight, np.float32)
    bands = np.zeros((128, KW * MB), np.float32)
    m = np.arange(STRIP)
    for dj in range(KW):
        for di in range(KH):
            bands[m + di, dj * MB + m] = w[di, dj]
    return bands.astype(np.float16)


def _prepare_in_maps(x, weight, bias):
    x16 = np.asarray(x, np.float32).astype(np.float16)
    bands = _build_bands(weight)
    bias_tile = np.full((128, 1), np.float32(np.asarray(bias).reshape(-1)[0]))

    in_maps = []
    for c in range(N_CORES):
        c0 = c * CW
        avail = min(IW, W - c0)
        if avail == IW:
            xs = np.ascontiguousarray(x16[:, c0 : c0 + IW])
        else:
            xs = np.zeros((H, IW), np.float16)
            xs[:, :avail] = x16[:, c0 : c0 + avail]
        in_maps.append({"xs": xs, "bands": bands, "biasv": bias_tile})
    return in_maps


def _gather_out(per_core_outs) -> np.ndarray:
    out = np.empty((OH, OW), np.float32)
    for c in range(N_CORES):
        c0 = c * CW
        take = min(CW, OW - c0)
        out[:, c0 : c0 + take] = per_core_outs[c]["out"][:, :take].astype(np.float32)
    return out


def kernel(x: np.ndarray, weight: np.ndarray, bias: np.ndarray) -> np.ndarray:
    from concourse import bass_utils

    nc = _get_nc()
    in_maps = _prepare_in_maps(x, weight, bias)
    res = bass_utils.run_bass_kernel_spmd(nc, in_maps, list(range(N_CORES)))
    _cache["last_results"] = res
    return _gather_out(res.results)


# revision 3
# speedup vs baseline: 2.0788x; 1.1015x over previous
"""Trainium2 Bass kernel: 7x7 valid cross-correlation + bias on a 4096x4096 f32 image.

Formulation: banded matmul on the TensorEngine.
  out[r, c] = sum_{di,dj} w[di,dj] * x[r+di, c+dj]
For an output row-strip of M=122 rows starting at r0, using K=128 input rows:
  out[r0+m, c] = sum_k A_dj[k, m] * x[r0+k, c+dj]   summed over dj=0..6
where A_dj[k, m] = w[k-m, dj] for 0 <= k-m < 7 (a banded [128, 122] matrix,
precomputed on host from the 49 kernel weights). The 7 dj-terms accumulate
into one PSUM bank via shifted column slices of the same SBUF rhs tile.

All matmul operands are fp16 (1 cycle/row on the PE vs fp32's 4), PSUM
accumulates fp32, output staged to SBUF as fp16 and upcast on the host.

DMA strategy: each dma_start costs ~1.2us of sequencer time regardless of
size, and PE stalls reset the p-state ramp (2.4GHz only after 3us of
continuous execution). So the host pre-arranges the input strip-major as
xst[p, s, c] = x[122*s + p, c0 + c] and the kernel loads it in a few large
chunk DMAs on the SP queue; outputs are written strip-major to
outt[m, s, c] = out[122*s + m, c0 + c] in one batched DMA per weight-group
on the Activation queue. The PE then streams matmuls back-to-back.

Sharding: output columns are split across the 8 cores (512 cols/core);
each core processes all 4090 output rows. Kernel + bias replicated.
"""

import numpy as np

H, W = 4096, 4096
KH, KW = 7, 7
OH, OW = H - KH + 1, W - KW + 1  # 4090, 4090
N_CORES = 8
CW = 512               # output columns per core
IW = CW + KW - 1       # input columns per core (518)
STRIP = 122            # output rows per strip (K = STRIP + KH - 1 = 128)
MB = 128               # stationary block columns (M padded 122 -> 128)
N_STRIPS = (OH + STRIP - 1) // STRIP  # 34 (last strip M=64, K=70)
G = 4                  # strips per weight-group / output batch
CHUNK = 8              # strips per input DMA chunk

_cache = {}


def _build_nc():
    import concourse.bacc as bacc
    import concourse.mybir as mybir
    from concourse.tile import TileContext

    f16 = mybir.dt.float16
    f32 = mybir.dt.float32

    n_chunks = (N_STRIPS + CHUNK - 1) // CHUNK

    nc = bacc.Bacc("TRN2", target_bir_lowering=False, debug=False)
    xst = nc.dram_tensor("xst", [128, N_STRIPS, IW], f16, kind="ExternalInput")
    bands = nc.dram_tensor("bands", [128, KW * MB], f16, kind="ExternalInput")
    biasv = nc.dram_tensor("biasv", [128, 1], f32, kind="ExternalInput")
    outt = nc.dram_tensor("outt", [STRIP, N_STRIPS, CW], f16, kind="ExternalOutput")

    with TileContext(nc) as tc:
        with (
            tc.tile_pool(name="const", bufs=1) as cpool,
            tc.tile_pool(name="rhs", bufs=n_chunks) as rpool,
            tc.tile_pool(name="obuf", bufs=2) as opool,
            tc.tile_pool(name="psum", bufs=8, space="PSUM") as ppool,
        ):
            band_t = cpool.tile([128, KW * MB], f16)
            nc.scalar.dma_start(out=band_t[:, :], in_=bands[:, :])
            bias_t = cpool.tile([128, 1], f32)
            nc.scalar.dma_start(out=bias_t[:, :], in_=biasv[:, :])

            chunk_tiles = []
            for ci in range(n_chunks):
                s0 = ci * CHUNK
                ns = min(CHUNK, N_STRIPS - s0)
                ct = rpool.tile([128, CHUNK * IW], f16, tag="rhs")
                nc.sync.dma_start(
                    out=ct[:, : ns * IW], in_=xst[:, s0 : s0 + ns, :]
                )
                chunk_tiles.append(ct)

            for g0 in range(0, N_STRIPS, G):
                strips = list(range(g0, min(g0 + G, N_STRIPS)))
                ng = len(strips)
                ps_ts, dims = [], []
                for s in strips:
                    r0 = s * STRIP
                    K = min(128, H - r0)
                    dims.append(K)
                    ps_ts.append(ppool.tile([128, CW], f32, name="ps", tag="ps"))
                for dj in range(KW):
                    lhsT = band_t[:, dj * MB : dj * MB + MB]
                    for s, ps, K in zip(strips, ps_ts, dims):
                        ct = chunk_tiles[s // CHUNK]
                        off = (s % CHUNK) * IW
                        nc.tensor.matmul(
                            ps[:, :],
                            lhsT[:K, :],
                            ct[:K, off + dj : off + dj + CW],
                            start=(dj == 0),
                            stop=(dj == KW - 1),
                        )
                ot = opool.tile([128, G * CW], f16, tag="ot")
                for i, ps in enumerate(ps_ts):
                    nc.vector.tensor_scalar_add(
                        ot[:STRIP, i * CW : (i + 1) * CW],
                        ps[:STRIP, :],
                        bias_t[:STRIP, :1],
                    )
                nc.scalar.dma_start(
                    out=outt[:, g0 : g0 + ng, :], in_=ot[:STRIP, : ng * CW]
                )

    nc.finalize()
    return nc


def _get_nc():
    if "nc" not in _cache:
        _cache["nc"] = _build_nc()
    return _cache["nc"]


def _build_bands(weight: np.ndarray) -> np.ndarray:
    """bands[k, dj*MB + m] = weight[k - m, dj] for 0 <= k-m < KH, m < STRIP."""
    w = np.asarray(weight, np.float32)
    bands = np.zeros((128, KW * MB), np.float32)
    m = np.arange(STRIP)
    for dj in range(KW):
        for di in range(KH):
            bands[m + di, dj * MB + m] = w[di, dj]
    return bands.astype(np.float16)


def _prepare_in_maps(x, weight, bias):
    x16 = np.asarray(x, np.float32).astype(np.float16)
    bands = _build_bands(weight)
    bias_tile = np.full((128, 1), np.float32(np.asarray(bias).reshape(-1)[0]))

    # padded copy: rows up to 122*33+127, cols up to 7*512+517
    rmax = STRIP * (N_STRIPS - 1) + 128
    cmax = CW * (N_CORES - 1) + IW
    xp = np.zeros((rmax, cmax), np.float16)
    xp[:H, :W] = x16
    rows = STRIP * np.arange(N_STRIPS)[None, :] + np.arange(128)[:, None]  # [128, S]

    in_maps = []
    for c in range(N_CORES):
        c0 = c * CW
        blk = xp[:, c0 : c0 + IW]          # [rmax, IW]
        xst = np.ascontiguousarray(blk[rows])  # [128, S, IW]
        in_maps.append({"xst": xst, "bands": bands, "biasv": bias_tile})
    return in_maps


def _gather_out(per_core_outs) -> np.ndarray:
    out = np.empty((OH, OW), np.float32)
    for c in range(N_CORES):
        c0 = c * CW
        take = min(CW, OW - c0)
        ot = per_core_outs[c]["outt"]  # [STRIP, S, CW] fp16
        rows = ot.transpose(1, 0, 2).reshape(N_STRIPS * STRIP, CW)[:OH]
        out[:, c0 : c0 + take] = rows[:, :take].astype(np.float32)
    return out


def kernel(x: np.ndarray, weight: np.ndarray, bias: np.ndarray) -> np.ndarray:
    from concourse import bass_utils

    nc = _get_nc()
    in_maps = _prepare_in_maps(x, weight, bias)
    res = bass_utils.run_bass_kernel_spmd(nc, in_maps, list(range(N_CORES)))
    _cache["last_results"] = res
    return _gather_out(res.results)


# revision 8
# speedup vs baseline: 2.2882x; 1.1008x over previous
"""Trainium2 Bass kernel: 7x7 valid cross-correlation + bias on a 4096x4096 f32 image.

Formulation: banded matmul on the TensorEngine.
  out[r, c] = sum_{di,dj} w[di,dj] * x[r+di, c+dj]
For an output row-strip of M=122 rows starting at r0, using K=128 input rows:
  out[r0+m, c] = sum_k A_dj[k, m] * x[r0+k, c+dj]   summed over dj=0..6
where A_dj[k, m] = w[k-m, dj] for 0 <= k-m < 7 (a banded [128, 122] matrix,
precomputed on host from the 49 kernel weights). The 7 dj-terms accumulate
into one PSUM bank via shifted column slices of the same SBUF rhs tile.

All matmul operands are fp16 (1 cycle/row on the PE vs fp32's 4), PSUM
accumulates fp32, output staged to SBUF as fp16 and upcast on the host.

DMA strategy: each dma_start costs ~1.2us of sequencer time regardless of
size, and PE stalls reset the p-state ramp (2.4GHz only after 3us of
continuous execution). So the host pre-arranges the input strip-major as
xst[p, s, c] = x[122*s + p, c0 + c] and the kernel loads it in a few large
chunk DMAs on the SP queue; outputs are written strip-major to
outt[m, s, c] = out[122*s + m, c0 + c] in one batched DMA per weight-group
on the Activation queue. The PE then streams matmuls back-to-back.

Sharding: output columns are split across the 8 cores (512 cols/core);
each core processes all 4090 output rows. Kernel + bias replicated.
"""

import numpy as np

H, W = 4096, 4096
KH, KW = 7, 7
OH, OW = H - KH + 1, W - KW + 1  # 4090, 4090
N_CORES = 8
CW = 512               # output columns per core
IW = CW + KW - 1       # input columns per core (518)
STRIP = 122            # output rows per strip (K = STRIP + KH - 1 = 128)
MB = 128               # stationary block columns (M padded 122 -> 128)
N_STRIPS = (OH + STRIP - 1) // STRIP  # 34 (last strip M=64, K=70)
G = 4                  # strips per weight-group / output batch
CHUNK_SIZES = (2, 6, 8, 9, 9)  # input DMA chunks: small first chunk = early start

_cache = {}


def _chunks():
    s0 = 0
    out = []
    for ns in CHUNK_SIZES:
        ns = min(ns, N_STRIPS - s0)
        if ns <= 0:
            break
        out.append((s0, ns))
        s0 += ns
    assert s0 == N_STRIPS, (s0, N_STRIPS)
    return out


def _build_nc():
    import concourse.bacc as bacc
    import concourse.mybir as mybir
    from concourse.tile import TileContext

    f16 = mybir.dt.float16
    f32 = mybir.dt.float32

    n_chunks = len(_chunks())

    nc = bacc.Bacc("TRN2", target_bir_lowering=False, debug=False)
    xst = nc.dram_tensor("xst", [128, N_STRIPS, IW], f16, kind="ExternalInput")
    bands = nc.dram_tensor("bands", [128, KW * MB], f16, kind="ExternalInput")
    biasv = nc.dram_tensor("biasv", [128, 1], f32, kind="ExternalInput")
    outt = nc.dram_tensor("outt", [STRIP, N_STRIPS, CW], f16, kind="ExternalOutput")

    with TileContext(nc) as tc:
        with (
            tc.tile_pool(name="const", bufs=1) as cpool,
            tc.tile_pool(name="rhs", bufs=n_chunks) as rpool,
            tc.tile_pool(name="obuf", bufs=4) as opool,
            tc.tile_pool(name="psum", bufs=8, space="PSUM") as ppool,
        ):
            band_t = cpool.tile([128, KW * MB], f16)
            nc.scalar.dma_start(out=band_t[:, :], in_=bands[:, :])
            bias_t = cpool.tile([128, 1], f32)
            nc.scalar.dma_start(out=bias_t[:, :], in_=biasv[:, :])

            chunk_map = {}
            for ci, (s0, ns) in enumerate(_chunks()):
                ct = rpool.tile([128, ns * IW], f16, tag=f"rhs{ci}")
                nc.sync.dma_start(
                    out=ct[:, : ns * IW], in_=xst[:, s0 : s0 + ns, :]
                )
                for i in range(ns):
                    chunk_map[s0 + i] = (ct, i * IW)

            for g0 in range(0, N_STRIPS, G):
                strips = list(range(g0, min(g0 + G, N_STRIPS)))
                ng = len(strips)
                ps_ts, dims = [], []
                for s in strips:
                    r0 = s * STRIP
                    K = min(128, H - r0)
                    dims.append(K)
                    ps_ts.append(ppool.tile([128, CW], f32, name="ps", tag="ps"))
                for dj in range(KW):
                    lhsT = band_t[:, dj * MB : dj * MB + MB]
                    for s, ps, K in zip(strips, ps_ts, dims):
                        ct, off = chunk_map[s]
                        nc.tensor.matmul(
                            ps[:, :],
                            lhsT[:K, :],
                            ct[:K, off + dj : off + dj + CW],
                            start=(dj == 0),
                            stop=(dj == KW - 1),
                        )
                ot = opool.tile([128, G * CW], f16, tag="ot")
                for i, ps in enumerate(ps_ts):
                    nc.vector.tensor_scalar_add(
                        ot[:STRIP, i * CW : (i + 1) * CW],
                        ps[:STRIP, :],
                        bias_t[:STRIP, :1],
                    )
                qeng = (nc.scalar, nc.sync, nc.gpsimd)[(g0 // G) % 3]
                qeng.dma_start(
                    out=outt[:, g0 : g0 + ng, :], in_=ot[:STRIP, : ng * CW]
                )

    nc.finalize()
    return nc


def _get_nc():
    if "nc" not in _cache:
        _cache["nc"] = _build_nc()
    return _cache["nc"]


def _build_bands(weight: np.ndarray) -> np.ndarray:
    """bands[k, dj*MB + m] = weight[k - m, dj] for 0 <= k-m < KH, m < STRIP."""
    w = np.asarray(weight, np.float32)
    bands = np.zeros((128, KW * MB), np.float32)
    m = np.arange(STRIP)
    for dj in range(KW):
        for di in range(KH):
            bands[m + di, dj * MB + m] = w[di, dj]
    return bands.astype(np.float16)


def _prepare_in_maps(x, weight, bias):
    x16 = np.asarray(x, np.float32).astype(np.float16)
    bands = _build_bands(weight)
    bias_tile = np.full((128, 1), np.float32(np.asarray(bias).reshape(-1)[0]))

    # padded copy: rows up to 122*33+127, cols up to 7*512+517
    rmax = STRIP * (N_STRIPS - 1) + 128
    cmax = CW * (N_CORES - 1) + IW
    xp = np.zeros((rmax, cmax), np.float16)
    xp[:H, :W] = x16
    rows = STRIP * np.arange(N_STRIPS)[None, :] + np.arange(128)[:, None]  # [128, S]

    in_maps = []
    for c in range(N_CORES):
        c0 = c * CW
        blk = xp[:, c0 : c0 + IW]          # [rmax, IW]
        xst = np.ascontiguousarray(blk[rows])  # [128, S, IW]
        in_maps.append({"xst": xst, "bands": bands, "biasv": bias_tile})
    return in_maps


def _gather_out(per_core_outs) -> np.ndarray:
    out = np.empty((OH, OW), np.float32)
    for c in range(N_CORES):
        c0 = c * CW
        take = min(CW, OW - c0)
        ot = per_core_outs[c]["outt"]  # [STRIP, S, CW] fp16
        rows = ot.transpose(1, 0, 2).reshape(N_STRIPS * STRIP, CW)[:OH]
        out[:, c0 : c0 + take] = rows[:, :take].astype(np.float32)
    return out


def kernel(x: np.ndarray, weight: np.ndarray, bias: np.ndarray) -> np.ndarray:
    from concourse import bass_utils

    nc = _get_nc()
    in_maps = _prepare_in_maps(x, weight, bias)
    res = bass_utils.run_bass_kernel_spmd(nc, in_maps, list(range(N_CORES)))
    _cache["last_results"] = res
    return _gather_out(res.results)


# revision 9
# speedup vs baseline: 2.8936x; 1.2646x over previous
"""Trainium2 Bass kernel: 7x7 valid cross-correlation + bias on a 4096x4096 f32 image.

Formulation: banded matmul on the TensorEngine.
  out[r, c] = sum_{di,dj} w[di,dj] * x[r+di, c+dj]
For an output row-strip of M=122 rows starting at r0, using K=128 input rows:
  out[r0+m, c] = sum_k A_dj[k, m] * x[r0+k, c+dj]   summed over dj=0..6
where A_dj[k, m] = w[k-m, dj] for 0 <= k-m < 7 (a banded [128, 122] matrix,
precomputed on host from the 49 kernel weights). The 7 dj-terms accumulate
into one PSUM bank via shifted column slices of the same SBUF rhs tile.

All matmul operands are fp16 (1 cycle/row on the PE vs fp32's 4), PSUM
accumulates fp32, output staged to SBUF as fp16 and upcast on the host.

DMA strategy: each dma_start costs ~1.2us of sequencer time regardless of
size, and PE stalls reset the p-state ramp (2.4GHz only after 3us of
continuous execution). So the host pre-arranges the input strip-major as
xst[p, s, c] = x[122*s + p, c0 + c] and the kernel loads it in a few large
chunk DMAs on the SP queue; outputs are written strip-major to
outt[m, s, c] = out[122*s + m, c0 + c] in one batched DMA per weight-group
on the Activation queue. The PE then streams matmuls back-to-back.

Sharding: output columns are split across the 8 cores (512 cols/core);
each core processes all 4090 output rows. Kernel + bias replicated.
"""

import numpy as np

H, W = 4096, 4096
KH, KW = 7, 7
OH, OW = H - KH + 1, W - KW + 1  # 4090, 4090
N_CORES = 8
CW = 512               # output columns per core
IW = CW + KW - 1       # input columns per core (518)
STRIP = 122            # output rows per strip (K = STRIP + KH - 1 = 128)
MB = 128               # stationary block columns (M padded 122 -> 128)
N_STRIPS = (OH + STRIP - 1) // STRIP  # 34 (last strip M=64, K=70)
G = 4                  # strips per weight-group / output batch
CHUNK_SIZES = (2, 6, 8, 9, 9)  # input DMA chunks: small first chunk = early start

_cache = {}


def _chunks():
    s0 = 0
    out = []
    for ns in CHUNK_SIZES:
        ns = min(ns, N_STRIPS - s0)
        if ns <= 0:
            break
        out.append((s0, ns))
        s0 += ns
    assert s0 == N_STRIPS, (s0, N_STRIPS)
    return out


def _build_nc():
    import concourse.bacc as bacc
    import concourse.mybir as mybir
    from concourse.tile import TileContext

    f16 = mybir.dt.float16
    f32 = mybir.dt.float32

    n_chunks = len(_chunks())

    nc = bacc.Bacc("TRN2", target_bir_lowering=False, debug=False)
    xst = nc.dram_tensor("xst", [128, N_STRIPS, IW], f16, kind="ExternalInput")
    bands = nc.dram_tensor("bands", [128, KW * MB], f16, kind="ExternalInput")
    biasv = nc.dram_tensor("biasv", [128, 1], f32, kind="ExternalInput")
    outt = nc.dram_tensor("outt", [STRIP, N_STRIPS, CW], f16, kind="ExternalOutput")

    with TileContext(nc) as tc:
        with (
            tc.tile_pool(name="const", bufs=1) as cpool,
            tc.tile_pool(name="rhs", bufs=n_chunks) as rpool,
            tc.tile_pool(name="obuf", bufs=4) as opool,
            tc.tile_pool(name="psum", bufs=8, space="PSUM") as ppool,
        ):
            band_t = cpool.tile([128, KW * MB], f16)
            nc.scalar.dma_start(out=band_t[:, :], in_=bands[:, :])
            bias_t = cpool.tile([128, 1], f32)
            nc.scalar.dma_start(out=bias_t[:, :], in_=biasv[:, :])

            chunk_map = {}
            for ci, (s0, ns) in enumerate(_chunks()):
                ct = rpool.tile([128, ns * IW], f16, tag=f"rhs{ci}")
                nc.sync.dma_start(
                    out=ct[:, : ns * IW], in_=xst[:, s0 : s0 + ns, :]
                )
                for i in range(ns):
                    chunk_map[s0 + i] = (ct, i * IW)

            for g0 in range(0, N_STRIPS, G):
                strips = list(range(g0, min(g0 + G, N_STRIPS)))
                ng = len(strips)
                ps_ts, dims = [], []
                for s in strips:
                    r0 = s * STRIP
                    K = min(128, H - r0)
                    dims.append(K)
                    ps_ts.append(ppool.tile([128, CW], f32, name="ps", tag="ps"))
                for dj in range(KW):
                    lhsT = band_t[:, dj * MB : dj * MB + MB]
                    for s, ps, K in zip(strips, ps_ts, dims):
                        ct, off = chunk_map[s]
                        nc.tensor.matmul(
                            ps[:, :],
                            lhsT[:K, :],
                            ct[:K, off + dj : off + dj + CW],
                            start=(dj == 0),
                            stop=(dj == KW - 1),
                        )
                ot = opool.tile([128, G * CW], f16, tag="ot")
                for i, ps in enumerate(ps_ts):
                    nc.vector.tensor_scalar_add(
                        ot[:STRIP, i * CW : (i + 1) * CW],
                        ps[:STRIP, :],
                        bias_t[:STRIP, :1],
                    )
                nc.gpsimd.dma_start(
                    out=outt[:, g0 : g0 + ng, :], in_=ot[:STRIP, : ng * CW]
                )

    nc.finalize()
    return nc


def _get_nc():
    if "nc" not in _cache:
        _cache["nc"] = _build_nc()
    return _cache["nc"]


def _build_bands(weight: np.ndarray) -> np.ndarray:
    """bands[k, dj*MB + m] = weight[k - m, dj] for 0 <= k-m < KH, m < STRIP."""
    w = np.asarray(weight, np.float32)
    bands = np.zeros((128, KW * MB), np.float32)
    m = np.arange(STRIP)
    for dj in range(KW):
        for di in range(KH):
            bands[m + di, dj * MB + m] = w[di, dj]
    return bands.astype(np.float16)


def _prepare_in_maps(x, weight, bias):
    x16 = np.asarray(x, np.float32).astype(np.float16)
    bands = _build_bands(weight)
    bias_tile = np.full((128, 1), np.float32(np.asarray(bias).reshape(-1)[0]))

    # padded copy: rows up to 122*33+127, cols up to 7*512+517
    rmax = STRIP * (N_STRIPS - 1) + 128
    cmax = CW * (N_CORES - 1) + IW
    xp = np.zeros((rmax, cmax), np.float16)
    xp[:H, :W] = x16
    rows = STRIP * np.arange(N_STRIPS)[None, :] + np.arange(128)[:, None]  # [128, S]

    in_maps = []
    for c in range(N_CORES):
        c0 = c * CW
        blk = xp[:, c0 : c0 + IW]          # [rmax, IW]
        xst = np.ascontiguousarray(blk[rows])  # [128, S, IW]
        in_maps.append({"xst": xst, "bands": bands, "biasv": bias_tile})
    return in_maps


def _gather_out(per_core_outs) -> np.ndarray:
    out = np.empty((OH, OW), np.float32)
    for c in range(N_CORES):
        c0 = c * CW
        take = min(CW, OW - c0)
        ot = per_core_outs[c]["outt"]  # [STRIP, S, CW] fp16
        rows = ot.transpose(1, 0, 2).reshape(N_STRIPS * STRIP, CW)[:OH]
        out[:, c0 : c0 + take] = rows[:, :take].astype(np.float32)
    return out


def kernel(x: np.ndarray, weight: np.ndarray, bias: np.ndarray) -> np.ndarray:
    from concourse import bass_utils

    nc = _get_nc()
    in_maps = _prepare_in_maps(x, weight, bias)
    res = bass_utils.run_bass_kernel_spmd(nc, in_maps, list(range(N_CORES)))
    _cache["last_results"] = res
    return _gather_out(res.results)
